# revision 13
# baseline (speedup 1.0000x reference)
"""Trainium2 Bass kernel for nn_Attention (B=2, L=2048, D=1024, H=16 heads).

Sharding (8 cores): data-parallel over batch (2) x tensor-parallel over heads
(4 groups of 4 heads), Megatron-style. Each core computes, for its batch b and
its 4 heads:
    QT/KT = (Wq_s @ x^T)               (transposed-layout projections; 1/8 score
                                        scale pre-folded into Wq/bq on host)
    V     = x_v @ Wv_s^T               (natural layout; key-padding mask folded
                                        into x_v rows on host)
    S^T   = K Q^T per head             (keys on partitions, queries free)
    P^T   = exp(S^T) * causal_mask     (no row-max: scores are O(4))
    [y^T; sums] = [V | kmask]^T P^T    (AV matmul also folds softmax sums)
    y_norm^T = y^T * approx(1/sums)    (gpsimd partition-broadcast of 1/sums)
    out_partial = y_norm @ Wp_s^T      (column shard of Wp)
Host sums the 4 partials per batch and adds bp + bv @ Wp^T (exact: softmax rows
sum to 1, so the V-bias contribution is a constant row vector).

Perf structure: all matmuls bf16 (fp32 PSUM). The program interleaves, per
512-query chunk j: V/Q/K projection (PE-dense), then attention for both head
pairs (ACT-dense exp), then the output projection for the chunk — so the Tile
scheduler can overlap chunk j's softmax with chunk j+1's projections and the
PE never starves (keeps the HAM clock-gate warm). Each pair's S^T = K Q^T runs
as two concurrent row-tiled matmuls (heads at array rows 0-63 / 64-127, K=64
each) into two halves of one PSUM pair of banks.
"""

import numpy as np
import ml_dtypes
import concourse.bass as bass
import concourse.tile as tile
from concourse import bacc, mybir
from concourse.bass import ds, ts
from concourse.bass_utils import run_bass_kernel_spmd

F32 = mybir.dt.float32
BF16 = mybir.dt.bfloat16

B, L, D, H = 2, 2048, 1024, 16
HD = 64          # head dim
HPC = 4          # heads per core
DS = HPC * HD    # 256: per-core shard of D
P = 128
NCORES = 8
LT = L // P      # 16 l-tiles
NJ = L // 512    # 4 q-chunks
NDT = D // P     # 8 contraction tiles over D

_CACHE = {}


def _build():
    nc = bacc.Bacc("TRN2", target_bir_lowering=False, debug=False, num_devices=NCORES)

    xq = nc.declare_dram_parameter("xq", [D, L], BF16, isOutput=False)
    xk = nc.declare_dram_parameter("xk", [D, L], BF16, isOutput=False)
    xv = nc.declare_dram_parameter("xv", [D, L], BF16, isOutput=False)
    wq = nc.declare_dram_parameter("wq", [P, NDT, DS], BF16, isOutput=False)
    wk = nc.declare_dram_parameter("wk", [P, NDT, DS], BF16, isOutput=False)
    wv = nc.declare_dram_parameter("wv", [P, NDT, DS], BF16, isOutput=False)
    wp = nc.declare_dram_parameter("wp", [P, 2, D], BF16, isOutput=False)
    bqp = nc.declare_dram_parameter("bq", [P, 2], F32, isOutput=False)
    bkp = nc.declare_dram_parameter("bk", [P, 2], F32, isOutput=False)
    emask = nc.declare_dram_parameter("emask", [P, LT], BF16, isOutput=False)
    cmask = nc.declare_dram_parameter("cmask", [P, 4, 2, 512], BF16, isOutput=False)
    out = nc.declare_dram_parameter("out", [L, D], BF16, isOutput=True)

    with tile.TileContext(nc) as tc:
        with tc.tile_pool(name="consts", bufs=1) as consts, \
             tc.tile_pool(name="stp", bufs=1) as stp, \
             tc.tile_pool(name="xp", bufs=48) as xp, \
             tc.tile_pool(name="ptp", bufs=8) as ptp, \
             tc.tile_pool(name="rp", bufs=2) as rp, \
             tc.tile_pool(name="op", bufs=2) as op, \
             tc.tile_pool(name="pp", bufs=4, space="PSUM") as pp:

            # ---- constants ----
            wq_sb = consts.tile([P, NDT, DS], BF16)
            wk_sb = consts.tile([P, NDT, DS], BF16)
            wv_sb = consts.tile([P, NDT, DS], BF16)
            for (wsb_, wdr_) in ((wv_sb, wv), (wq_sb, wq), (wk_sb, wk)):
                for dt2 in range(0, NDT, 2):
                    nc.scalar.dma_start(wsb_[:, dt2:dt2 + 2, :], wdr_[:, dt2:dt2 + 2, :])
            wp_sb = consts.tile([P, 2, D], BF16)
            nc.scalar.dma_start(wp_sb[:], wp[:])
            bq_sb = consts.tile([P, 2], F32)
            bk_sb = consts.tile([P, 2], F32)
            nc.scalar.dma_start(bq_sb[:], bqp[:])
            nc.scalar.dma_start(bk_sb[:], bkp[:])
            em_sb = consts.tile([P, LT], BF16)
            nc.scalar.dma_start(em_sb[:], emask[:])
            cm_sb = consts.tile([P, 4, 2, 512], BF16)
            nc.scalar.dma_start(cm_sb[:], cmask[:])

            # ---- PE warm-up during input DMA lead-in (results never read) ----
            wu = consts.tile([P, 512], BF16)
            nc.any.memset(wu[:], 0.25)
            for i in range(32):
                pwu = pp.tile([P, 1024], F32, tag="pp")
                nc.tensor.matmul(pwu[:, 0:512], wu[:, 0:128], wu[:], start=True, stop=True)

            def outproj(jo):
                YTo = YTs[jo]
                for lt in range(4):
                    osb = op.tile([P, D], BF16, tag="o")
                    po = pp.tile([P, 1024], F32, tag="pp")
                    for dc in range(2):
                        for hc in range(2):
                            nc.tensor.matmul(po[:, ds(512 * dc, 512)],
                                             YTo[:, hc, ts(lt, P)],
                                             wp_sb[:, hc, ds(512 * dc, 512)],
                                             start=(hc == 0), stop=(hc == 1))
                    nc.scalar.copy(osb[:, 0:512], po[:, 0:512])
                    nc.vector.tensor_copy(osb[:, 512:1024], po[:, 512:1024])
                    nc.gpsimd.dma_start(out[ts(4 * jo + lt, P), :], osb[:])

            def fetch(j2):
                tl = {}
                for (nm, xin) in (("v", xv), ("q", xq), ("k", xk)):
                    tt = []
                    for dt in range(NDT):
                        t = xp.tile([P, 512], BF16, tag="x")
                        nc.sync.dma_start(t[:], xin[ds(P * dt, P), ds(512 * j2, 512)])
                        tt.append(t)
                    tl[nm] = tt
                return tl

            QTs, KTs, Vs, YTs = [], [], [], []
            pref = fetch(0)

            for j in range(NJ):
                # ---- V projection chunk j: V[l, dout] natural layout ----
                Vt = stp.tile([P, 4, HPC * (HD + 1)], BF16, tag=f"V{j}")
                Vs.append(Vt)
                for h in range(HPC):
                    col = 65 * h + HD
                    nc.scalar.copy(Vt[:, :, col:col + 1],
                                   em_sb[:, 4 * j:4 * j + 4, None])
                cur = pref
                xt = cur["v"]
                for sub in range(4):
                    pv = pp.tile([P, 1024], F32, tag="pp")
                    for dt in range(NDT):
                        nc.tensor.matmul(pv[:, 0:DS], xt[dt][:, ds(128 * sub, 128)],
                                         wv_sb[:, dt, :], start=(dt == 0),
                                         stop=(dt == NDT - 1))
                    nc.vector.tensor_copy(
                        Vt[:, sub, :].rearrange("p (h c) -> p h c", h=HPC)[:, :, 0:HD],
                        pv[:, 0:DS].rearrange("p (h c) -> p h c", h=HPC))

                # ---- Q/K projections chunk j ----
                QTt = stp.tile([P, 2, 512], BF16, tag=f"Q{j}")
                KTt = stp.tile([P, 2, 512], BF16, tag=f"K{j}")
                QTs.append(QTt)
                KTs.append(KTt)
                for (xt, wsb, bsb, dst) in ((cur["q"], wq_sb, bq_sb, QTt),
                                            (cur["k"], wk_sb, bk_sb, KTt)):
                    for dc in range(2):
                        pacc = pp.tile([P, 1024], F32, tag="pp")
                        for dt in range(NDT):
                            nc.tensor.matmul(pacc[:, 0:512], wsb[:, dt, ds(128 * dc, 128)],
                                             xt[dt][:], start=(dt == 0),
                                             stop=(dt == NDT - 1))
                        nc.vector.tensor_scalar_add(dst[:, dc, :], pacc[:, 0:512],
                                                    bsb[:, dc:dc + 1])

                if j + 1 < NJ:
                    pref = fetch(j + 1)

                # ---- attention chunk j, two head pairs ----
                # pair p = heads (2p, 2p+1): head A on partitions 0-63, head B
                # on 64-127 of QT/KT column group hc=p. S^T for both heads runs
                # as two concurrent row-tiled matmuls (K=64 each).
                YTt = stp.tile([P, 2, 512], BF16, tag=f"Y{j}")
                YTs.append(YTt)
                nk = 4 * j + 4
                for pair in range(2):
                    pyt = pp.tile([P, 1024], F32, tag="pp")
                    pytA = pyt[0:65, 0:512]
                    pytB = pyt[0:65, 512:1024]
                    for t in range(nk):
                        jc, st = divmod(t, 4)
                        # diagonal tiles t>=4j+2: q-cols [0:256) of this chunk
                        # are fully causal-masked -> skip computing them
                        trim = 256 if t - 4 * j >= 2 else 0
                        qw = 512 - trim
                        pwt = pp.tile([P, 1024], F32, tag="pp")
                        nc.tensor.matmul(pwt[:, ds(0, qw)],
                                         KTs[jc][0:64, pair, ds(128 * st, P)],
                                         QTt[0:64, pair, ds(trim, qw)],
                                         start=True, stop=True,
                                         tile_position=(0, 0))
                        nc.tensor.matmul(pwt[:, ds(512, qw)],
                                         KTs[jc][64:128, pair, ds(128 * st, P)],
                                         QTt[64:128, pair, ds(trim, qw)],
                                         start=True, stop=True,
                                         tile_position=(64, 0))
                        pt = ptp.tile([P, 1024], BF16, tag="pt")
                        pwv = pwt[:].rearrange("p (a b) -> p a b", a=2)[:, :, 0:qw]
                        ptv = pt[:, ds(0, 2 * qw)].rearrange("p (a b) -> p a b", a=2)
                        nc.scalar.activation(ptv, pwv,
                                             mybir.ActivationFunctionType.Exp)
                        r = t - 4 * j
                        if r >= 0:
                            nc.vector.tensor_mul(out=ptv, in0=ptv,
                                                 in1=cm_sb[:, r, :, trim:512])
                        for (a, pya) in ((0, pytA), (1, pytB)):
                            nc.tensor.matmul(pya[:, ds(trim, qw)],
                                             Vs[jc][:, st, ds(65 * (2 * pair + a), HD + 1)],
                                             pt[:, ds(a * qw, qw)],
                                             start=(t == 0), stop=(t == nk - 1))
                    # normalization: R = approx(1/sums), broadcast to the
                    # head's 64 partitions, scale y^T
                    rras, Rbs, Rrs = [], [], []
                    for (a, pya) in ((0, pytA), (1, pytB)):
                        rrow = rp.tile([P, 512], F32, tag=f"rr{a}")
                        nc.vector.tensor_copy(rrow[0:1, :], pya[64:65, :])
                        rras.append(rrow)
                    for a in range(2):
                        Rb = rp.tile([P, 512], F32, tag=f"rb{a}")
                        nc.gpsimd.partition_broadcast(Rb[:], rras[a][0:1, :])
                        Rbs.append(Rb)
                    for a in range(2):
                        Rr = rp.tile([P, 512], F32, tag=f"rc{a}")
                        nc.vector.reciprocal_approx_fast(Rr[:], Rbs[a][:])
                        Rrs.append(Rr)
                    for (a, pya) in ((0, pytA), (1, pytB)):
                        nc.vector.tensor_mul(out=YTt[ds(64 * a, HD), pair, :],
                                             in0=pya[0:64, :],
                                             in1=Rrs[a][ds(64 * a, HD), :])
                    if pair == 0 and j > 0:
                        outproj(j - 1)


            outproj(NJ - 1)

    nc.compile()
    return nc


def _get_nc():
    if "nc" not in _CACHE:
        _CACHE["nc"] = _build()
    return _CACHE["nc"]


def _wlayout(w):
    # [D, DS] -> [P, NDT, DS] with row (o*P + p) at [p, o, :]
    return np.ascontiguousarray(
        w.reshape(NDT, P, DS).transpose(1, 0, 2).astype(ml_dtypes.bfloat16))


def _wlayout2(w):
    # [DS, D] -> [P, 2, D]
    return np.ascontiguousarray(
        w.reshape(2, P, D).transpose(1, 0, 2).astype(ml_dtypes.bfloat16))


def _shard_inputs(query, key, value, kmask, Wq, bq, Wk, bk, Wv, Wp):
    kk = np.arange(P)[:, None, None]
    rr = np.arange(4)[None, :, None]
    qq = np.arange(512)[None, None, :]
    cm = (P * rr + kk <= qq).astype(ml_dtypes.bfloat16)  # [P, 4, 512]
    cmask = np.ascontiguousarray(
        np.broadcast_to(cm[:, :, None, :], (P, 4, 2, 512)))
    scale = 0.125  # 1/sqrt(HD), folded into the Q projection
    bf = ml_dtypes.bfloat16
    in_maps = []
    for c in range(NCORES):
        b, hg = divmod(c, HPC)
        hs = slice(DS * hg, DS * (hg + 1))
        kvalid = kmask[b].astype(np.float32)
        in_maps.append({
            "xq": np.ascontiguousarray(query[b].T.astype(bf)),
            "xk": np.ascontiguousarray(key[b].T.astype(bf)),
            "xv": np.ascontiguousarray((value[b] * kvalid[:, None]).T.astype(bf)),
            "wq": _wlayout(Wq[hs].T * scale),
            "wk": _wlayout(Wk[hs].T),
            "wv": _wlayout(Wv[hs].T),
            "wp": _wlayout2(Wp[:, hs].T),
            "bq": np.ascontiguousarray((bq[hs] * scale).reshape(2, P).T),
            "bk": np.ascontiguousarray(bk[hs].reshape(2, P).T),
            "emask": np.ascontiguousarray(kvalid.reshape(LT, P).T.astype(bf)),
            "cmask": cmask,
        })
    return in_maps


def kernel(query, key, value, kmask, Wq, bq, Wk, bk, Wv, bv, Wp, bp):
    query = np.asarray(query, dtype=np.float32)
    key = np.asarray(key, dtype=np.float32)
    value = np.asarray(value, dtype=np.float32)
    kmask = np.asarray(kmask)
    Wq = np.asarray(Wq, dtype=np.float32)
    bq = np.asarray(bq, dtype=np.float32)
    Wk = np.asarray(Wk, dtype=np.float32)
    bk = np.asarray(bk, dtype=np.float32)
    Wv = np.asarray(Wv, dtype=np.float32)
    bv = np.asarray(bv, dtype=np.float32)
    Wp = np.asarray(Wp, dtype=np.float32)
    bp = np.asarray(bp, dtype=np.float32)

    in_maps = _shard_inputs(query, key, value, kmask, Wq, bq, Wk, bk, Wv, Wp)
    nc = _get_nc()
    res = run_bass_kernel_spmd(nc, in_maps, list(range(NCORES))).results

    outp = np.zeros((B, L, D), dtype=np.float32)
    for c in range(NCORES):
        b = c // HPC
        outp[b] += res[c]["out"].astype(np.float32)
    outp += bp[None, None, :] + (bv @ Wp.T)[None, None, :]
    return outp
